# revision 7
# baseline (speedup 1.0000x reference)
"""Trainium2 Bass kernel for CentersDistance (vq_codebook).

logits[c, q] = -||centers[c] - inputs[q]||^2  for inputs [4096,128], centers [256,128].

Strategy (per spec sharding hint): shard inputs along Q across 8 cores
(512 queries/core), replicate centers. Each core computes its transposed
slab outT[q, c] = 2*dot(q,c) - ||c||^2 - ||q||^2 via TensorE matmuls:

  - load x-shard interleaved ([p,n,d], query 4p+n) for 2KB DMA descriptors
  - PE-transpose x (4 tiles) and centers (2 tiles) to D-major layout
  - qnorm via DVE square+reduce on the native layout (per-partition -> ACT bias)
  - cnorm via ones-matmul on cT^2, scaled -0.5, broadcast to a parked PSUM
    bank with a K=1 matmul (free-axis term -> DVE tensor_tensor add)
  - main matmuls: psum[q,c] = xT_n.T @ cT  (K=128, N=256, fp32)
  - epilogue: DVE add (dot - cn/2), ACT Identity(2*t + (-qnorm)) -> SBUF -> DMA

Host side: gather the 8 [512,256] slabs, transpose, concat -> [256,4096].
"""

import numpy as np
from contextlib import ExitStack

import concourse.bass as bass
import concourse.bacc as bacc
import concourse.tile as tile
from concourse import mybir
from concourse.bass_utils import run_bass_kernel_spmd
from concourse.masks import make_identity

Q, C, D = 4096, 256, 128
NCORES = 8
QL = Q // NCORES      # 512 queries per core
NQ = QL // 128        # 4 query chunks per core
NCT = C // 128        # 2 center chunks
F32 = mybir.dt.float32

_NC = None
LAST_RESULTS = None


def _build_nc():
    nc = bacc.Bacc("TRN2", target_bir_lowering=False)
    x = nc.declare_dram_parameter("x", [QL, D], F32, isOutput=False)
    cen = nc.declare_dram_parameter("c", [C, D], F32, isOutput=False)
    out = nc.declare_dram_parameter("out", [QL, C], F32, isOutput=True)

    mult = mybir.AluOpType.mult
    add = mybir.AluOpType.add
    IDENT = mybir.ActivationFunctionType.Identity

    with ExitStack() as ctx:
        tc = ctx.enter_context(tile.TileContext(nc))
        const = ctx.enter_context(tc.tile_pool(name="const", bufs=1))
        work = ctx.enter_context(tc.tile_pool(name="work", bufs=2))
        outp = ctx.enter_context(tc.tile_pool(name="outp", bufs=4))
        pt_pool = ctx.enter_context(
            tc.tile_pool(name="pt", bufs=2, space=bass.MemorySpace.PSUM)
        )
        pm_pool = ctx.enter_context(
            tc.tile_pool(name="pm", bufs=4, space=bass.MemorySpace.PSUM)
        )
        pc_pool = ctx.enter_context(
            tc.tile_pool(name="pc", bufs=1, space=bass.MemorySpace.PSUM)
        )

        ident = const.tile([128, 128], F32)
        make_identity(nc, ident[:])
        ones_col = const.tile([128, 1], F32)
        nc.gpsimd.memset(ones_col[:], 1.0)
        ones_row = const.tile([1, 128], F32)
        nc.gpsimd.memset(ones_row[:], 1.0)

        # x interleaved: partition p holds queries 4p+n, 2KB/partition DMA
        x_raw = const.tile([128, NQ, D], F32)
        nc.sync.dma_start(x_raw[:], x.rearrange("(p n) d -> p n d", n=NQ))
        # centers natural chunking: chunk n = centers[n*128:(n+1)*128]
        c_raw = const.tile([128, NCT, D], F32)
        nc.sync.dma_start(c_raw[:], cen.rearrange("(n p) d -> p n d", p=128))

        # -qnorm per partition/chunk: ACT Square with accum_out (free-axis sum),
        # then negate. (tensor_tensor_reduce crashes the exec unit on this HW.)
        qn_pos = const.tile([128, NQ], F32)
        for n in range(NQ):
            scr = work.tile([128, D], F32, tag="scr")
            nc.scalar.activation(
                scr[:], x_raw[:, n, :], mybir.ActivationFunctionType.Square,
                accum_out=qn_pos[:, n : n + 1],
            )
        qn = const.tile([128, NQ], F32)
        nc.vector.tensor_scalar_mul(qn[:], qn_pos[:], -1.0)

        # transpose centers -> cT [d, c]
        cT = const.tile([128, C], F32)
        for n in range(NCT):
            pt = pt_pool.tile([128, 128], F32, tag="pt")
            nc.tensor.transpose(pt[:], c_raw[:, n, :], ident[:])
            nc.scalar.copy(cT[:, bass.ts(n, 128)], pt[:])

        # transpose x chunks -> xT [d, q'] per chunk
        xT = const.tile([128, NQ, 128], F32)
        for n in range(NQ):
            pt = pt_pool.tile([128, 128], F32, tag="pt")
            nc.tensor.transpose(pt[:], x_raw[:, n, :], ident[:])
            nc.scalar.copy(xT[:, n, :], pt[:])

        # -cnorm as a row, broadcast across all partitions (GPSIMD)
        sq = work.tile([128, C], F32, tag="sq")
        nc.vector.tensor_mul(sq[:], cT[:], cT[:])
        cn_ps = pc_pool.tile([1, C], F32)
        nc.tensor.matmul(cn_ps[:], ones_col[:], sq[:], start=True, stop=True)
        crow = const.tile([1, C], F32)
        nc.scalar.mul(crow[:], cn_ps[:], -1.0)
        cb_ps = pc_pool.tile([128, C], F32)
        nc.tensor.matmul(cb_ps[:], ones_row[:], crow[:], start=True, stop=True)
        cb = const.tile([128, C], F32)
        nc.scalar.copy(cb[:], cb_ps[:])

        # mains + epilogue: out = (2*dot - qnorm) + (-cnorm)
        out3 = out.rearrange("(p n) c -> p n c", n=NQ)
        for n in range(NQ):
            ps = pm_pool.tile([128, C], F32, tag="ps")
            nc.tensor.matmul(ps[:], xT[:, n, :], cT[:], start=True, stop=True)
            t = work.tile([128, C], F32, tag="t")
            nc.scalar.activation(t[:], ps[:], IDENT, bias=qn[:, n : n + 1], scale=2.0)
            o = outp.tile([128, C], F32, tag="o")
            nc.vector.tensor_add(o[:], t[:], cb[:])
            nc.sync.dma_start(out3[:, n, :], o[:])

    nc.compile()  # Bacc register allocation; walrus rejects unallocated regs
    return nc


def get_nc():
    global _NC
    if _NC is None:
        _NC = _build_nc()
    return _NC


def kernel(inputs: np.ndarray, centers: np.ndarray, trace: bool = False):
    global LAST_RESULTS
    inputs = np.ascontiguousarray(np.asarray(inputs, dtype=np.float32))
    centers = np.ascontiguousarray(np.asarray(centers, dtype=np.float32))
    assert inputs.shape == (Q, D) and centers.shape == (C, D)

    nc = get_nc()
    in_maps = [
        {"x": inputs[i * QL : (i + 1) * QL], "c": centers} for i in range(NCORES)
    ]
    res = run_bass_kernel_spmd(nc, in_maps, list(range(NCORES)), trace=trace)
    LAST_RESULTS = res
    full = np.empty((C, Q), dtype=np.float32)
    for i in range(NCORES):
        full[:, i * QL : (i + 1) * QL] = res.results[i]["out"].T
    return full


# revision 8
# speedup vs baseline: 1.0430x; 1.0430x over previous
"""Trainium2 Bass kernel for CentersDistance (vq_codebook).

logits[c, q] = -||centers[c] - inputs[q]||^2  for inputs [4096,128], centers [256,128].

Strategy (per spec sharding hint): shard inputs along Q across 8 cores
(512 queries/core), replicate centers. Each core computes its transposed
slab outT[q, c] = 2*dot(q,c) - ||c||^2 - ||q||^2 via TensorE matmuls:

  - load x-shard interleaved ([p,n,d], query 4p+n) for 2KB DMA descriptors
  - PE-transpose x (4 tiles) and centers (2 tiles) into PSUM supertiles,
    one merged PSUM->SBUF copy each (xT gets scale=2 folded in)
  - qnorm: one DVE square + one negated tensor_reduce (native layout)
  - cnorm: ones-matmul on cT^2 -> row, scale -1, K=1 matmul broadcast (PSUM)
  - bias[p,n,c] = -qnorm[p,n] - cnorm[c]: one DVE add of two broadcast APs
  - mains: psum[q,c] = (2*xT_n).T @ cT  (K=128, N=256, fp32), 4 chunks into
    bank-aligned slices of one padded PSUM supertile
  - epilogue: one DVE add (psum + bias) -> SBUF -> one output DMA

Host side: gather the 8 [512,256] slabs, transpose, concat -> [256,4096].
"""

import numpy as np
from contextlib import ExitStack

import concourse.bass as bass
import concourse.bacc as bacc
import concourse.tile as tile
from concourse import mybir
from concourse.bass_utils import run_bass_kernel_spmd
from concourse.masks import make_identity

Q, C, D = 4096, 256, 128
NCORES = 8
QL = Q // NCORES      # 512 queries per core
NQ = QL // 128        # 4 query chunks per core
NCT = C // 128        # 2 center chunks
F32 = mybir.dt.float32

_NC = None
LAST_RESULTS = None


def _build_nc():
    nc = bacc.Bacc("TRN2", target_bir_lowering=False)
    x = nc.declare_dram_parameter("x", [QL, D], F32, isOutput=False)
    cen = nc.declare_dram_parameter("c", [C, D], F32, isOutput=False)
    out = nc.declare_dram_parameter("out", [QL, C], F32, isOutput=True)

    with ExitStack() as ctx:
        tc = ctx.enter_context(tile.TileContext(nc))
        const = ctx.enter_context(tc.tile_pool(name="const", bufs=1))
        work = ctx.enter_context(tc.tile_pool(name="work", bufs=1))
        ptx = ctx.enter_context(
            tc.tile_pool(name="ptx", bufs=1, space=bass.MemorySpace.PSUM)
        )
        ptc = ctx.enter_context(
            tc.tile_pool(name="ptc", bufs=1, space=bass.MemorySpace.PSUM)
        )
        pm = ctx.enter_context(
            tc.tile_pool(name="pm", bufs=1, space=bass.MemorySpace.PSUM)
        )
        pn = ctx.enter_context(
            tc.tile_pool(name="pn", bufs=1, space=bass.MemorySpace.PSUM)
        )

        ident = const.tile([128, 128], F32)
        make_identity(nc, ident[:])
        ones_col = const.tile([128, 1], F32)
        nc.gpsimd.memset(ones_col[:], 1.0)
        ones_row = const.tile([1, 128], F32)
        nc.gpsimd.memset(ones_row[:], 1.0)

        # x interleaved: partition p holds queries 4p+n (2KB/partition DMA)
        x_raw = const.tile([128, NQ, D], F32)
        nc.sync.dma_start(x_raw[:], x.rearrange("(p n) d -> p n d", n=NQ))
        # centers natural chunking: chunk n = centers[n*128:(n+1)*128]
        c_raw = const.tile([128, NCT, D], F32)
        nc.sync.dma_start(c_raw[:], cen.rearrange("(n p) d -> p n d", p=128))

        # -qnorm[p, n]: square + negated free-axis reduce (DVE)
        x2 = work.tile([128, NQ, D], F32)
        nc.vector.tensor_mul(x2[:], x_raw[:], x_raw[:])
        qn = const.tile([128, NQ, 1], F32)
        nc.vector.tensor_reduce(
            qn[:], x2[:], mybir.AxisListType.X, mybir.AluOpType.add, negate=True
        )

        # PE transposes into PSUM supertiles, one merged copy out of each
        T_x = ptx.tile([128, NQ, 128], F32)
        for n in range(NQ):
            nc.tensor.transpose(T_x[:, n, :], x_raw[:, n, :], ident[:])
        xT = const.tile([128, NQ, 128], F32)
        nc.scalar.mul(xT[:], T_x[:], 2.0)  # fold the *2 of 2*dot into x
        T_c = ptc.tile([128, NCT, 128], F32)
        for n in range(NCT):
            nc.tensor.transpose(T_c[:, n, :], c_raw[:, n, :], ident[:])
        cT = const.tile([128, C], F32)
        nc.scalar.copy(cT[:].rearrange("p (n d) -> p n d", n=NCT), T_c[:])

        # -cnorm row -> K=1 matmul broadcast across partitions (stays in PSUM)
        cT2 = work.tile([128, C], F32)
        nc.vector.tensor_mul(cT2[:], cT[:], cT[:])
        cn_ps = pn.tile([1, C], F32)
        nc.tensor.matmul(cn_ps[:], ones_col[:], cT2[:], start=True, stop=True)
        crow = const.tile([1, C], F32)
        nc.scalar.mul(crow[:], cn_ps[:], -1.0)
        cb_ps = pn.tile([128, 1, C], F32)
        nc.tensor.matmul(
            cb_ps[:, 0, :], ones_row[:], crow[:], start=True, stop=True
        )

        # bias[p,n,c] = -qnorm[p,n] - cnorm[c] (off the mains' critical path)
        bias = work.tile([128, NQ, C], F32)
        nc.vector.tensor_add(
            bias[:],
            qn[:].broadcast_to([128, NQ, C]),
            cb_ps[:].broadcast_to([128, NQ, C]),
        )

        # mains into bank-aligned slices of a padded PSUM supertile
        P = pm.tile([128, NQ, 512], F32)  # 512-pad -> each chunk owns a bank
        for n in range(NQ):
            nc.tensor.matmul(P[:, n, 0:C], xT[:, n, :], cT[:], start=True, stop=True)

        # epilogue: one big DVE add, one output DMA
        o = const.tile([128, NQ, C], F32)
        nc.vector.tensor_add(o[:], P[:, :, 0:C], bias[:])
        nc.sync.dma_start(out.rearrange("(p n) c -> p n c", n=NQ), o[:])

    nc.compile()  # Bacc register allocation; walrus rejects unallocated regs
    return nc


def get_nc():
    global _NC
    if _NC is None:
        _NC = _build_nc()
    return _NC


def kernel(inputs: np.ndarray, centers: np.ndarray, trace: bool = False):
    global LAST_RESULTS
    inputs = np.ascontiguousarray(np.asarray(inputs, dtype=np.float32))
    centers = np.ascontiguousarray(np.asarray(centers, dtype=np.float32))
    assert inputs.shape == (Q, D) and centers.shape == (C, D)

    nc = get_nc()
    in_maps = [
        {"x": inputs[i * QL : (i + 1) * QL], "c": centers} for i in range(NCORES)
    ]
    res = run_bass_kernel_spmd(nc, in_maps, list(range(NCORES)), trace=trace)
    LAST_RESULTS = res
    full = np.empty((C, Q), dtype=np.float32)
    for i in range(NCORES):
        full[:, i * QL : (i + 1) * QL] = res.results[i]["out"].T
    return full
